# revision 2
# baseline (speedup 1.0000x reference)
"""Sliding-window (causal band) multi-head attention on 8 Trainium2 cores.

Problem (hardcoded): B=2, N=2048, dim=1024, H=16, Dh=64, window=256.
  qkv = x @ W_qkv; rotary(q, k); scores = q k^T / 8 with causal band mask
  (q-256 <= k <= q); out = softmax(scores) @ v @ W_out.

Sharding: sequence-parallel. 8 cores = (batch b in 2) x (quarter qr in 4);
each core owns 512 tokens of one batch and receives a 768-token frame
(256-token halo before its chunk; zero-padded for qr=0). Each core
recomputes k/v for its halo locally, so there is no cross-core traffic.
Host feeds x pre-transposed (feature-major) per core; outputs come back
feature-major [1024, 512] and the host transposes/concatenates.

On-core layout is feature-major throughout (dim on partitions, tokens on
the free axis), which makes every matmul a full-speed (moving dim >= 256,
fp32r) PE op and needs no on-chip transposes:
  q^T/k^T:  [128 = 2 heads x 64, tokens]   (fp32r, rotary applied on DVE)
  scores^T: [k-tokens, q-tokens] blocks of the 3-diagonal band structure
  attn@v:   lhsT = [v | kvalid] (bf16, M=65) -> out^T plus the softmax
            denominator in partition 64 of the same PSUM tile
  out-proj: lhsT = W_out tiles, rhs = normalized head outputs.
"""

import numpy as np

HEADS = 16
DH = 64
WIN = 256
B = 2
N = 2048
D = 1024
CHUNK = 512          # tokens owned per core
F = CHUNK + WIN      # 768-token frame (halo + own)
NCORES = 8

# q-window (in local q coords 0..512) covered by each of the 6 k-subtiles;
# widened to >=256 so fp32r matmuls stay at full speed.
SWIN = [(0, 256), (0, 256), (0, 384), (128, 512), (256, 512), (256, 512)]

_cache = {}


def _build_program():
    import concourse.bacc as bacc
    import concourse.mybir as mybir
    import concourse.tile as tile

    f32 = mybir.dt.float32
    f32r = mybir.dt.float32r
    bf16 = mybir.dt.bfloat16
    Exp = mybir.ActivationFunctionType.Exp

    nc = bacc.Bacc("TRN2", target_bir_lowering=False, debug=False,
                   num_devices=NCORES)

    xT_d = nc.dram_tensor("xT", [D, F], f32r, kind="ExternalInput").ap()
    cosT_d = nc.dram_tensor("cosT", [DH, F], f32, kind="ExternalInput").ap()
    sinT_d = nc.dram_tensor("sinT", [DH, F], f32, kind="ExternalInput").ap()
    wqkv_d = nc.dram_tensor("W_qkv", [D, 3 * D], f32r, kind="ExternalInput").ap()
    wout_d = nc.dram_tensor("W_out", [D, D], f32r, kind="ExternalInput").ap()
    kv_d = nc.dram_tensor("kvalid", [128, 6], f32, kind="ExternalInput").ap()
    mlo_d = nc.dram_tensor("mask_lo", [128, 128], bf16, kind="ExternalInput").ap()
    mhi_d = nc.dram_tensor("mask_hi", [128, 128], bf16, kind="ExternalInput").ap()
    yT_d = nc.dram_tensor("yT", [D, CHUNK], f32, kind="ExternalOutput").ap()

    with tile.TileContext(nc) as tc:
        import contextlib

        stack = contextlib.ExitStack()
        with stack:
            pers = stack.enter_context(tc.tile_pool(name="pers", bufs=1))
            scratch = stack.enter_context(tc.tile_pool(name="scratch", bufs=2))
            wpool = stack.enter_context(tc.tile_pool(name="w", bufs=4))

            xT = pers.tile([128, 8, F], f32r)
            for d in range(8):
                nc.sync.dma_start(out=xT[:, d, :], in_=xT_d[128 * d:128 * (d + 1), :])
            cos2 = pers.tile([128, F], f32)
            sin2 = pers.tile([128, F], f32)
            nc.sync.dma_start(out=cos2[0:64, :], in_=cosT_d)
            nc.sync.dma_start(out=cos2[64:128, :], in_=cosT_d)
            nc.sync.dma_start(out=sin2[0:64, :], in_=sinT_d)
            nc.sync.dma_start(out=sin2[64:128, :], in_=sinT_d)
            mlo = pers.tile([128, 128], bf16)
            mhi = pers.tile([128, 128], bf16)
            nc.sync.dma_start(out=mlo, in_=mlo_d)
            nc.sync.dma_start(out=mhi, in_=mhi_d)
            kval = pers.tile([128, 6], f32)
            nc.sync.dma_start(out=kval, in_=kv_d)

            q_sb = pers.tile([128, 8, CHUNK], f32r)
            k_sb = pers.tile([128, 8, F], f32r)
            v_all = pers.tile([128, 6, HEADS, DH + 1], bf16)
            oh_sb = pers.tile([128, 8, CHUNK], f32r)

            def rotary_into(dst, psum, w0, w1):
                # dst = psum * cos + rotate_half(psum) * sin  over cols [w0, w1)
                w = w1 - w0
                plain = scratch.tile([128, F], f32, tag="rot_plain")
                sh = scratch.tile([128, F], f32, tag="rot_sh")
                t1 = scratch.tile([128, F], f32, tag="rot_t1")
                nc.scalar.copy(plain[:, :w], psum)
                for g in range(4):
                    s = g ^ 1
                    nc.sync.dma_start(out=sh[g * 32:(g + 1) * 32, :w],
                                      in_=plain[s * 32:(s + 1) * 32, :w])
                nc.vector.tensor_mul(t1[:, :w], plain[:, :w], cos2[:, w0:w1])
                nc.vector.tensor_mul(sh[:, :w], sh[:, :w], sin2[:, w0:w1])
                nc.vector.tensor_add(dst, t1[:, :w], sh[:, :w])

            with tc.tile_pool(name="psum_proj", bufs=4, space="PSUM") as psumP:
                # ---- Q projection (own 512 tokens only) + rotary ----
                for cg in range(4):
                    pq = [psumP.tile([128, CHUNK], f32, tag="proj", name=f"pq{cg}_{e}")
                          for e in range(2)]
                    for d in range(8):
                        w = wpool.tile([128, 256], f32r, tag="wqk")
                        nc.sync.dma_start(
                            out=w, in_=wqkv_d[128 * d:128 * (d + 1),
                                              256 * cg:256 * (cg + 1)])
                        for e in range(2):
                            nc.tensor.matmul(pq[e][:], w[:, 128 * e:128 * (e + 1)],
                                             xT[:, d, WIN:F],
                                             start=(d == 0), stop=(d == 7))
                    for e in range(2):
                        rotary_into(q_sb[:, 2 * cg + e, :], pq[e][:], WIN, F)

                # ---- K projection (all 768 tokens, two 384-windows) + rotary ----
                for cg in range(4):
                    pk = [[psumP.tile([128, 384], f32, tag="proj",
                                       name=f"pk{cg}_{e}_{win}")
                           for win in range(2)] for e in range(2)]
                    for d in range(8):
                        w = wpool.tile([128, 256], f32r, tag="wqk")
                        nc.sync.dma_start(
                            out=w, in_=wqkv_d[128 * d:128 * (d + 1),
                                              D + 256 * cg:D + 256 * (cg + 1)])
                        for e in range(2):
                            for win in range(2):
                                nc.tensor.matmul(
                                    pk[e][win][:], w[:, 128 * e:128 * (e + 1)],
                                    xT[:, d, 384 * win:384 * (win + 1)],
                                    start=(d == 0), stop=(d == 7))
                    for e in range(2):
                        for win in range(2):
                            rotary_into(k_sb[:, 2 * cg + e,
                                             384 * win:384 * (win + 1)],
                                        pk[e][win][:], 384 * win, 384 * (win + 1))

                # ---- V projection (x^T stationary -> token-major v) ----
                for hh in range(2):
                    wvs = []
                    for d in range(8):
                        w = wpool.tile([128, 512], f32r, tag="wv", bufs=9)
                        nc.sync.dma_start(
                            out=w, in_=wqkv_d[128 * d:128 * (d + 1),
                                              2 * D + 512 * hh:2 * D + 512 * (hh + 1)])
                        wvs.append(w)
                    for t in range(6):
                        pv = psumP.tile([128, 512], f32, tag="proj")
                        for d in range(8):
                            nc.tensor.matmul(pv[:], xT[:, d, 128 * t:128 * (t + 1)],
                                             wvs[d][:], start=(d == 0), stop=(d == 7))
                        nc.scalar.copy(
                            v_all[:, t, 8 * hh:8 * (hh + 1), 0:DH],
                            pv[:].rearrange("p (h e) -> p h e", h=8))
                for t in range(6):
                    nc.vector.tensor_copy(
                        v_all[:, t, :, DH:DH + 1],
                        kval[:, t:t + 1].to_broadcast([128, HEADS, 1]))

            # ---- attention, per pair of heads (hp = coltile) ----
            with (
                tc.tile_pool(name="psum_s", bufs=3, space="PSUM") as psumS,
                tc.tile_pool(name="psum_o", bufs=5, space="PSUM") as psumO,
                tc.tile_pool(name="expp", bufs=26) as expp,
            ):
                for hp in range(8):
                    exps = {}
                    for i in range(6):
                        w0, w1 = SWIN[i]
                        wd = w1 - w0
                        for hs in range(2):
                            pb = 64 * hs
                            ps = psumS.tile([128, 384], f32, tag="ps_s")
                            nc.tensor.matmul(
                                ps[:, :wd],
                                k_sb[pb:pb + 64, hp, 128 * i:128 * (i + 1)],
                                q_sb[pb:pb + 64, hp, w0:w1],
                                start=True, stop=True)
                            ex = expp.tile([128, 384], bf16, tag="ex")
                            nc.scalar.activation(ex[:, :wd], ps[:, :wd], Exp,
                                                 scale=0.125)
                            if i <= 3:  # lower-triangular block (j == i)
                                off = 128 * i - w0
                                nc.vector.tensor_mul(ex[:, off:off + 128],
                                                     ex[:, off:off + 128], mlo[:])
                            if i >= 2:  # upper-triangular block (j == i - 2)
                                off = 128 * (i - 2) - w0
                                nc.vector.tensor_mul(ex[:, off:off + 128],
                                                     ex[:, off:off + 128], mhi[:])
                            exps[(hs, i)] = ex

                    for hs in range(2):
                        g = 2 * hp + hs
                        recip = scratch.tile([128, CHUNK], f32, tag="recip")
                        pos = []
                        for j in range(4):
                            po = psumO.tile([65, 128], f32, tag="ps_o")
                            for n, i in enumerate((j, j + 1, j + 2)):
                                off = 128 * j - SWIN[i][0]
                                nc.tensor.matmul(
                                    po[:], v_all[:, i, g, :],
                                    exps[(hs, i)][:, off:off + 128],
                                    start=(n == 0), stop=(n == 2))
                            nc.vector.reciprocal(
                                recip[64:65, 128 * j:128 * (j + 1)], po[64:65, :])
                            pos.append(po)
                        r0 = scratch.tile([1, CHUNK], f32, tag="r0")
                        nc.vector.tensor_copy(r0[0:1, :], recip[64:65, :])
                        bc = scratch.tile([64, CHUNK], f32, tag="bc")
                        nc.gpsimd.partition_broadcast(bc[:], r0[0:1, :])
                        for j in range(4):
                            nc.vector.tensor_mul(
                                oh_sb[64 * hs:64 * (hs + 1), hp,
                                      128 * j:128 * (j + 1)],
                                pos[j][0:64, :], bc[:, 128 * j:128 * (j + 1)])

            # ---- output projection ----
            with tc.tile_pool(name="psum_y", bufs=4, space="PSUM") as psumY:
                for og in range(4):
                    py = [psumY.tile([128, CHUNK], f32, tag="ps_y", name=f"py{og}_{e}")
                          for e in range(2)]
                    for hp in range(8):
                        w = wpool.tile([128, 256], f32r, tag="wo")
                        nc.sync.dma_start(
                            out=w, in_=wout_d[128 * hp:128 * (hp + 1),
                                              256 * og:256 * (og + 1)])
                        for e in range(2):
                            nc.tensor.matmul(py[e][:], w[:, 128 * e:128 * (e + 1)],
                                             oh_sb[:, hp, :],
                                             start=(hp == 0), stop=(hp == 7))
                    for e in range(2):
                        o = 2 * og + e
                        ysb = scratch.tile([128, CHUNK], f32, tag="y")
                        nc.scalar.copy(ysb[:], py[e][:])
                        nc.sync.dma_start(out=yT_d[128 * o:128 * (o + 1), :],
                                          in_=ysb[:])

    nc.compile()
    return nc


def shard_inputs(x, rotary_emb, W_qkv, W_out):
    import ml_dtypes

    x = np.asarray(x, dtype=np.float32)
    rotary_emb = np.asarray(rotary_emb, dtype=np.float32)
    W_qkv = np.ascontiguousarray(np.asarray(W_qkv, dtype=np.float32))
    W_out = np.ascontiguousarray(np.asarray(W_out, dtype=np.float32))

    cos = np.cos(rotary_emb)                     # [N, 64]
    sin = np.sin(rotary_emb).copy()
    sin[:, :32] *= -1.0                          # sign-folded for rotate_half
    # padded [WIN + N, *] frames so every core slices uniformly
    xp = np.concatenate([np.zeros((B, WIN, D), np.float32), x], axis=1)
    cosp = np.concatenate([np.zeros((WIN, DH), np.float32), cos], axis=0)
    sinp = np.concatenate([np.zeros((WIN, DH), np.float32), sin], axis=0)

    mask_lo = np.tril(np.ones((128, 128), np.float32))   # keep r >= c
    mask_hi = np.triu(np.ones((128, 128), np.float32))   # keep r <= c
    mask_lo = mask_lo.astype(ml_dtypes.bfloat16)
    mask_hi = mask_hi.astype(ml_dtypes.bfloat16)

    in_maps = []
    for c in range(NCORES):
        b, qr = divmod(c, 4)
        lo = CHUNK * qr                         # frame start in padded coords
        kvalid = np.ones((F,), np.float32)
        if qr == 0:
            kvalid[:WIN] = 0.0
        in_maps.append({
            "xT": np.ascontiguousarray(xp[b, lo:lo + F, :].T),
            "cosT": np.ascontiguousarray(cosp[lo:lo + F, :].T),
            "sinT": np.ascontiguousarray(sinp[lo:lo + F, :].T),
            "W_qkv": W_qkv,
            "W_out": W_out,
            "kvalid": np.ascontiguousarray(kvalid.reshape(6, 128).T),
            "mask_lo": mask_lo,
            "mask_hi": mask_hi,
        })
    return in_maps


def unshard(results):
    out = np.empty((B, N, D), dtype=np.float32)
    for c, r in enumerate(results):
        b, qr = divmod(c, 4)
        out[b, CHUNK * qr:CHUNK * (qr + 1), :] = r["yT"].T
    return out


def kernel(x, rotary_emb, W_qkv, W_out):
    from concourse.bass_utils import run_bass_kernel_spmd

    if "nc" not in _cache:
        _cache["nc"] = _build_program()
    nc = _cache["nc"]
    in_maps = shard_inputs(x, rotary_emb, W_qkv, W_out)
    res = run_bass_kernel_spmd(nc, in_maps, core_ids=list(range(NCORES)),
                               trace=False)
    return unshard(res.results)
